# revision 2
# baseline (speedup 1.0000x reference)
"""Distillation loss (chunked KL + CE) on 8 Trainium2 NeuronCores — v4.

Probe-driven design (see probe.py phases 1-9). On this axon terminal the
HW behaves nothing like the instruction cost model:

  * instruction slots cost ~17-100us each (load-dependent), roughly
    serialized PER CORE across all engines; streaming afterwards runs at
    ~210-260 G elem/s, so wide ops are nearly free beyond the slot cost;
  * DMA costs ~0.2-0.9us per descriptor (one per contiguous run per
    partition, 64KB max per MAX_DMA_LAST_DIM).  The old kernel's
    8192 x 16KB descriptors are exactly its 5.9ms baseline.

So v4 minimizes total instruction slots and maximizes descriptor size:

  * fp8(e3m4) transport, host pre-scales logits by 1/4 so every device
    intermediate ((t-s)/4, e*(t-s)/4) stays inside fp8 range (+-15.5);
  * host concatenates the two logit tensors into ts=[2, 512, 32000] so ONE
    dma_start per half-shard loads both: dst [128, {tb,sb}, 64000] with
    2 x 64KB descriptors per partition (512 descriptors/core total);
  * SBUF arena [128, 192KB] = tb | eb | sb (tb,eb adjacent so one
    tensor_reduce covers both);
  * per half-shard (256 tokens = [128 part, 2 tok, 32000 vocab]):
      ACT A1 : eb = exp(0.8*tb)            (e_t; 0.8*(t/4) = t/5)
      ACT A3a/A3b: exp(4*sb[:,j,:]) accum -> Zce[tok j] (sink out)
      ACT A2 : eb = exp(0.8*sb)            (e_s)
      GP  G1 : tb = tb - sb                (D)
      GP  G2 : tb = tb * eb                (P = e_t*D)
      DVE R1 : reduce(tb|eb [128,16,8000]) -> [W(8) | Zu(8)]  (one instr!)
      DVE R2 : reduce(eb    [128, 8,8000]) -> Zv(8)
  * host combines per-core [128,52] f32 stats in float64:
      kl = 4*W/(T*Zu) + log Zv - log Zu   per (token, chunk)
      ce from Zce and s[label] (label gather on host, as before);
  * semaphore waits minimized via transitive implication; single waits
    fuse into the next instruction (this walrus only rejects multi-wait
    instructions).

Raw Bass with explicit engine blocks.
"""

from contextlib import ExitStack

import numpy as np

import concourse.bass as bass
import concourse.mybir as mybir
from concourse.bass_utils import run_bass_kernel_spmd

ALPHA = 0.7
TEMP = 5.0
PAD_ID = 0
NUM_CHUNKS = 4

N_CORES = 8
B, S, V = 2, 2048, 32000
TOK = B * S                      # 4096 tokens total
TPC = TOK // N_CORES             # 512 tokens per core
P = 128                          # SBUF partitions
TPP = 2                          # tokens per partition per tile
HALVES = TPC // (P * TPP)        # 2 half-shards per core
CHW = V // NUM_CHUNKS            # 8000
PRESCALE = 0.25                  # host multiplies logits by this before fp8

F8 = mybir.dt.float8e3           # e3m4: max 15.5, fits all intermediates
F32 = mybir.dt.float32
EXP = mybir.ActivationFunctionType.Exp
MULT = mybir.AluOpType.mult
SUB = mybir.AluOpType.subtract
ADD = mybir.AluOpType.add
X = mybir.AxisListType.X

WF = TPP * V                     # 64000 elems per partition per buffer
NSTAT = 26                       # per half: W[8] Zu[8] Zv[8] Zce[2]


def _build_nc(repeat=1):
    """Per-core program over ts=[2, TPC, V] fp8 (pre-scaled by 1/4)."""
    nc = bass.Bass()
    ts = nc.dram_tensor("ts", [2, TPC, V], F8, kind="ExternalInput")
    st = nc.dram_tensor("stats", [P, 2 * NSTAT], F32, kind="ExternalOutput")

    with ExitStack() as ctx:
        arena = ctx.enter_context(nc.sbuf_tensor("arena", [P, 3 * WF], F8))
        tb = arena[:, 0:WF]
        eb = arena[:, WF:2 * WF]
        sb = arena[:, 2 * WF:3 * WF]
        acc = ctx.enter_context(nc.sbuf_tensor("acc", [P, 2 * NSTAT], F32))
        sink = ctx.enter_context(nc.sbuf_tensor("sink", [P, 4], F32))
        dTS = ctx.enter_context(nc.semaphore("dTS"))  # +16 per merged load
        aE = ctx.enter_context(nc.semaphore("aE"))    # +4 per iter (ACT)
        gP = ctx.enter_context(nc.semaphore("gP"))    # +2 per iter (GPSIMD)
        vD = ctx.enter_context(nc.semaphore("vD"))    # +2 per iter (DVE)
        out_sem = ctx.enter_context(nc.semaphore("out_sem"))
        block = ctx.enter_context(nc.Block())

        niter = HALVES * repeat

        @block.sync
        def _(sync):
            for it in range(niter):
                r0 = (it % HALVES) * P * TPP
                if it > 0:
                    # tb free after R1(it-1); sb free after A2(it-1) (A2's
                    # wait on vD>=2it-1 also implies G1/A3 are long done).
                    sync.wait_ge(vD, 2 * it - 1)
                    sync.wait_ge(aE, 4 * it)
                # One DMA loads t-half into tb and s-half into sb:
                # src [a, (p j), v]; dst arena slices 0 (tb) and 2 (sb).
                src = ts[:, r0:r0 + P * TPP, :].rearrange(
                    "a (p j) v -> p a (j v)", p=P)
                dst = arena[:].rearrange("p (b x) -> p b x", b=3)[:, 0:3:2, :]
                sync.dma_start(out=dst, in_=src).then_inc(dTS, 16)
            sync.wait_ge(vD, 2 * niter)
            sync.wait_ge(aE, 4 * niter)
            sync.dma_start(out=st[:, :], in_=acc[:]).then_inc(out_sem, 16)
            sync.wait_ge(out_sem, 16)

        @block.scalar
        def _(scalar):
            for it in range(niter):
                o = NSTAT * (it % HALVES)
                # A1: eb = exp(0.8*tb).  eb free after R2(it-1) (which also
                # transitively implies A2/G2(it-1) completed).
                scalar.wait_ge(dTS, 16 * (it + 1))
                if it > 0:
                    scalar.wait_ge(vD, 2 * it)
                nc.scalar.activation(
                    eb, tb, EXP, bias=0.0, scale=0.8,
                ).then_inc(aE, 1)
                # A3a/A3b: Zce[tok j] = sum exp(4*sb[:, j, :])
                nc.scalar.activation(
                    sink[:, 0:1].broadcast_to([P, V]),
                    sb[:, 0:V], EXP, bias=0.0, scale=4.0,
                    accum_out=acc[:, o + 24:o + 25],
                ).then_inc(aE, 1)
                nc.scalar.activation(
                    sink[:, 1:2].broadcast_to([P, V]),
                    sb[:, V:2 * V], EXP, bias=0.0, scale=4.0,
                    accum_out=acc[:, o + 25:o + 26],
                ).then_inc(aE, 1)
                # A2: eb = exp(0.8*sb) after R1 (implies G2 freed eb)
                scalar.wait_ge(vD, 2 * it + 1)
                nc.scalar.activation(
                    eb, sb, EXP, bias=0.0, scale=0.8,
                ).then_inc(aE, 1)

        @block.gpsimd
        def _(gp):
            for it in range(niter):
                # G1: tb = tb - sb  (A1 done reading tb; implies loads done)
                gp.wait_ge(aE, 4 * it + 1)
                nc.gpsimd.tensor_tensor(
                    out=tb, in0=tb, in1=sb, op=SUB,
                ).then_inc(gP, 1)
                # G2: tb = tb * eb  (P = e_t * D)
                nc.gpsimd.tensor_tensor(
                    out=tb, in0=tb, in1=eb, op=MULT,
                ).then_inc(gP, 1)

        @block.vector
        def _(vector):
            for it in range(niter):
                o = NSTAT * (it % HALVES)
                # R1: one reduce over tb|eb -> [W(8) | Zu(8)]  (after G2,
                # which implies A1)
                vector.wait_ge(gP, 2 * it + 2)
                nc.vector.tensor_reduce(
                    out=acc[:, o + 0:o + 16],
                    in_=arena[:, 0:2 * WF].rearrange(
                        "p (m x) -> p m x", x=CHW),
                    axis=X, op=ADD,
                ).then_inc(vD, 1)
                # R2: Zv chunk-sums of e_s (after A2)
                vector.wait_ge(aE, 4 * (it + 1))
                nc.vector.tensor_reduce(
                    out=acc[:, o + 16:o + 24],
                    in_=eb.rearrange("p (m x) -> p m x", x=CHW),
                    axis=X, op=ADD,
                ).then_inc(vD, 1)

    return nc


_NC_CACHE = {}
last_results = None


def _get_nc(repeat=1):
    if repeat not in _NC_CACHE:
        _NC_CACHE[repeat] = _build_nc(repeat)
    return _NC_CACHE[repeat]


def _combine(results, s_full, lab):
    """Host-side float64 reduction of per-core [128, 52] stats -> loss."""
    # token index = core*TPC + 256*h + 2*p + j
    zu = np.empty((TOK, NUM_CHUNKS))
    w = np.empty((TOK, NUM_CHUNKS))
    zv = np.empty((TOK, NUM_CHUNKS))
    zce = np.empty(TOK)
    for c, r in enumerate(results):
        a = r["stats"].astype(np.float64)          # [128, 52]
        for h in range(HALVES):
            o = NSTAT * h
            base = c * TPC + P * TPP * h
            w_h = a[:, o + 0:o + 8].reshape(P, TPP, NUM_CHUNKS)
            zu_h = a[:, o + 8:o + 16].reshape(P, TPP, NUM_CHUNKS)
            zv_h = a[:, o + 16:o + 24].reshape(P, TPP, NUM_CHUNKS)
            zce_h = a[:, o + 24:o + 26]            # [128, 2]
            idx = (base + 2 * np.arange(P)[:, None]
                   + np.arange(TPP)[None, :]).ravel()
            w[idx] = w_h.reshape(-1, NUM_CHUNKS)
            zu[idx] = zu_h.reshape(-1, NUM_CHUNKS)
            zv[idx] = zv_h.reshape(-1, NUM_CHUNKS)
            zce[idx] = zce_h.ravel()

    # W stored = sum e_t*(t-s)/4 -> true sum e_t*(t-s) = 4*W
    kl = (4.0 * w) / (TEMP * zu) + np.log(zv) - np.log(zu)
    total_kl = kl.sum() * (TEMP * TEMP) * (CHW / V) / B

    s_label = s_full[np.arange(TOK), lab].astype(np.float64)
    nll = np.log(zce) - s_label
    valid = lab != PAD_ID
    n_valid = max(int(valid.sum()), 1)
    ce = float(nll[valid].sum()) / n_valid

    return ALPHA * total_kl + (1.0 - ALPHA) * ce


def kernel(student_logits, teacher_logits, labels):
    global last_results
    np_f8 = mybir.dt.np(F8)
    s_full = np.asarray(student_logits, dtype=np.float32).reshape(TOK, V)
    t_full = np.asarray(teacher_logits, dtype=np.float32).reshape(TOK, V)
    lab = np.asarray(labels).reshape(TOK).astype(np.int64)
    s_f8 = (s_full * PRESCALE).astype(np_f8)
    t_f8 = (t_full * PRESCALE).astype(np_f8)

    nc = _get_nc()
    in_maps = []
    for c in range(N_CORES):
        ts = np.ascontiguousarray(np.stack(
            [t_f8[c * TPC:(c + 1) * TPC], s_f8[c * TPC:(c + 1) * TPC]], axis=0))
        in_maps.append({"ts": ts})
    last_results = run_bass_kernel_spmd(nc, in_maps, core_ids=list(range(N_CORES)))
    loss = _combine(last_results.results, s_full, lab)
    return np.array(loss, dtype=np.float32)
